# revision 45
# baseline (speedup 1.0000x reference)
"""AdaFocalLoss on 8 Trainium2 NeuronCores (Bass/Tile, SPMD).

Data-parallel over the batch axis, per the sharding hint: each core gets
8192 of the 65536 logit rows, the 15-entry gamma table is replicated, and
the per-core partial sums are combined on the host (the gather/unshard
step; the reduction over rows is order-independent).

Per-core kernel structure:
  - Rows are assigned to (slot, partition) SORTED BY TARGET on the host:
    slot s holds the 128 rows whose targets sit near the s-th quantile of
    the target distribution.  The row order is free to choose (the final
    loss is a sum over rows), and sorting makes the target-logit gather
    cheap: all 128 targets of a slot fall inside a static 64-column
    window around the slot's quantile center.
  - The shard streams as 64 contiguous 512 KB DMAs (one per slot).
  - ScalarE computes exp(x) for every element (fp16 out; the only engine
    with transcendentals).  The per-row sum of exps comes from the
    ACTIVATE's accum_out for 28 of the 64 slots and from a VectorE
    tensor_scalar cache-reduce over the exp tile for the other 36 - the
    split balances the two engines' busy time (~95 us each, right at the
    ~92 us HBM roofline for the 33 MB/core of logits).
  - The target logit x_t is gathered on VectorE in one pass per slot:
    scalar_tensor_tensor  (iota == target_p) * x  with accum_out, scanned
    only over the slot's 64-column window.
  - Tail per row ([128, 64] values): lse = ln(sumexp), logpt = x_t - lse,
    pt = exp(logpt); gamma's sign s and magnitude m are looked up via a
    telescoped sum_b(delta_b * [pt >= b/15]) computed with broadcast-AP
    tensor ops; loss = -(1 + eps - s*pt)^m * logpt via exp(m*ln(u)).
    The tail runs in two unequal parts (48/16 slots) so only the small
    final part is exposed past the stream.
  - Per-partition row sums are reduced across partitions with a PE
    matmul against a ones vector; each core emits one f32 scalar.

The gather windows are data-independent quantile bands (+-32 columns
~ 5.8 sigma of the sampling deviation for iid targets; the reference
distribution measures a max deviation of 26).  If an unusual
target distribution ever falls outside them, the host check catches it
and the kernel transparently rebuilds with full-width windows (slower
but always correct).
"""

import sys

for _p in ("/opt/trn_rl_repo",):
    if _p not in sys.path:
        sys.path.insert(0, _p)

import numpy as np

NUM_BINS = 15
EPS = 1e-20
N, C = 65536, 1000
NCORES = 8
NSHARD = N // NCORES  # 8192 rows per core
P = 128  # SBUF partitions
R = NSHARD // P  # 64 row-slots per partition
W = 64  # gather window width (columns) per row-slot
TAIL_BOUNDS = [0, 48, 64]  # unequal tail parts: only the small last one is exposed
NPART = len(TAIL_BOUNDS) - 1
ACT_ACC = 28  # row-sums accumulated on ScalarE (the rest go to VectorE)
PAIRED = False  # pairing measured slower: wide ACTIVATEs run at a worse per-element rate
EO_F16 = True  # dtype of the exp scratch tile


def _slot_lo(w):
    # static window starts: slot s is centered on the s-th target quantile
    return [min(max(int(C * (s + 0.5) / R) - w // 2, 0), C - w) for s in range(R)]


def _split_excess_waits(nc, mybir, max_waits=1):
    """This container's walrus supports only one sync-wait command per
    instruction; hoist extra waits onto preceding same-engine no-ops."""
    ctr = 0
    for f in nc.m.functions:
        for bb in f.blocks:
            new_insts = []
            changed = False
            for inst in bb.instructions:
                si = inst.sync_info
                if si is not None and si.on_wait and len(si.on_wait) > max_waits:
                    waits = list(si.on_wait)
                    excess, keep = waits[:-max_waits], waits[-max_waits:]
                    for i in range(0, len(excess), max_waits):
                        ctr += 1
                        new_insts.append(
                            mybir.InstNoOp(
                                name=f"I-waitsplit-{ctr}",
                                sync_info=mybir.SyncInfo(
                                    on_wait=list(excess[i : i + max_waits]),
                                    on_update=[],
                                ),
                                bass_nofuse=True,
                                engine=inst.engine,
                            )
                        )
                    si.on_wait = keep
                    changed = True
                new_insts.append(inst)
            if changed:
                bb.instructions[:] = new_insts


def _build(w):
    import concourse.bass as bass
    import concourse.tile as tile
    from concourse import mybir

    f32 = mybir.dt.float32
    f16 = mybir.dt.float16 if EO_F16 else mybir.dt.float32
    AF = mybir.ActivationFunctionType
    ALU = mybir.AluOpType
    NB = NUM_BINS
    slot_lo = _slot_lo(w)

    nc = bass.Bass()
    x = nc.declare_dram_parameter("x", [NSHARD, C], f32, isOutput=False)
    tmap = nc.declare_dram_parameter("tmap", [P, R], f32, isOutput=False)
    iota = nc.declare_dram_parameter("iota", [P, C], f32, isOutput=False)
    gb = nc.declare_dram_parameter("gb", [P, NB], f32, isOutput=False)
    out = nc.declare_dram_parameter("out", [1, 1], f32, isOutput=True)

    # target-sorted rank-major layout: HBM row s*128 + p holds the row for
    # slot s, partition p, so each slot is one contiguous 512 KB DMA
    x3 = x[:].rearrange("(s p) c -> s p c", s=R, p=P)
    # paired view: u-th pair = slots (2u, 2u+1) in one [P, 2, C] transfer
    x4 = x[:].rearrange("(u q p) c -> u p q c", u=R // 2, q=2, p=P)

    # slots whose row-sum of exps is accumulated on ScalarE (cheap marginal
    # cost) vs VectorE (ts cache-reduce), spread evenly for smooth overlap
    if PAIRED:
        # groups of 4: [A A D D] x14 then [D D D D] x2 -> 28 ACT slots and
        # 18 adjacent D-pairs that share one DMA and one wide ACTIVATE
        act_slots = set()
        for g in range(R // 4):
            if g < 14:
                act_slots.add(4 * g)
                act_slots.add(4 * g + 1)
        schedule = []
        for g in range(R // 4):
            base = 4 * g
            if g < 14:
                schedule += [("A", base), ("A", base + 1), ("D2", base + 2)]
            else:
                schedule += [("D2", base), ("D2", base + 2)]
    else:
        act_slots = set(
            s for s in range(R) if (s * ACT_ACC) // R != ((s + 1) * ACT_ACC) // R
        )
        schedule = [("A" if s in act_slots else "D", s) for s in range(R)]

    def slot_part(slot):
        h = 0
        while slot >= TAIL_BOUNDS[h + 1]:
            h += 1
        return h, slot - TAIL_BOUNDS[h]

    part_w = [TAIL_BOUNDS[h + 1] - TAIL_BOUNDS[h] for h in range(NPART)]

    with tile.TileContext(nc) as tc:
        with (
            tc.tile_pool(name="const", bufs=1) as cpool,
            tc.tile_pool(name="io", bufs=8) as iopool,
            tc.tile_pool(name="escr", bufs=3) as epool,
            tc.tile_pool(name="sscr", bufs=3) as spool,
            tc.tile_pool(name="acc", bufs=1) as apool,
            tc.tile_pool(name="tail", bufs=3) as tpool,
            tc.tile_pool(name="psum", bufs=1, space="PSUM") as ppool,
        ):
            # a few row-slots stream before the constant loads so compute
            # can begin immediately
            early = {}
            for s in range(2):
                et = iopool.tile([P, C], f32, tag="xtile", name=f"xtile_e{s}")
                nc.sync.dma_start(et[:], x3[s, :, :])
                early[s] = et

            iota_t = cpool.tile([P, C], f32, tag="iota")
            nc.sync.dma_start(iota_t[:], iota[:])
            tmap_t = cpool.tile([P, R], f32, tag="tmap")
            nc.sync.dma_start(tmap_t[:], tmap[:])
            gb_t = cpool.tile([P, NB], f32, tag="gb")
            nc.sync.dma_start(gb_t[:], gb[:])

            # gamma sign/magnitude tables and their telescoped deltas:
            # g(bin(pt)) = sum_b dg_b * [pt >= b/15]
            sgn = cpool.tile([P, NB], f32, tag="sgn")
            nc.scalar.activation(sgn[:], gb_t[:], AF.Sign)
            mag = cpool.tile([P, NB], f32, tag="mag")
            nc.scalar.activation(mag[:], gb_t[:], AF.Abs)
            ds = cpool.tile([P, NB], f32, tag="ds")
            nc.vector.tensor_copy(ds[:, 0:1], sgn[:, 0:1])
            nc.vector.tensor_sub(ds[:, 1:NB], sgn[:, 1:NB], sgn[:, 0 : NB - 1])
            dm = cpool.tile([P, NB], f32, tag="dm")
            nc.vector.tensor_copy(dm[:, 0:1], mag[:, 0:1])
            nc.vector.tensor_sub(dm[:, 1:NB], mag[:, 1:NB], mag[:, 0 : NB - 1])
            # bin thresholds b/15, derived from the iota constant
            thr = cpool.tile([P, NB], f32, tag="thr")
            nc.vector.tensor_scalar(
                thr[:], iota_t[:, 0:NB], 1.0 / NB, None, ALU.mult
            )

            # per-half accumulators so each tail half only depends on its
            # own half of the main loop
            sumexp = [
                apool.tile([P, part_w[h]], f32, tag=f"sumexp{h}", name=f"sumexp{h}")
                for h in range(NPART)
            ]
            xt = [
                apool.tile([P, part_w[h]], f32, tag=f"xt{h}", name=f"xt{h}")
                for h in range(NPART)
            ]
            rowsums = []

            def tail_half(h):
                se, xh = sumexp[h], xt[h]
                F = part_w[h]
                lse = tpool.tile([P, F], f32, tag="lse")
                nc.scalar.activation(lse[:], se[:], AF.Ln)
                logpt = tpool.tile([P, F], f32, tag="logpt")
                nc.vector.tensor_sub(logpt[:], xh[:], lse[:])
                pt = tpool.tile([P, F], f32, tag="pt")
                nc.scalar.activation(pt[:], logpt[:], AF.Exp)

                # s(pt), m(pt) via broadcast APs: ge[p,j,b] = pt[p,j]>=thr[p,b]
                ge = tpool.tile([P, F * NB], f32, tag="ge")
                ge3 = ge[:].rearrange("p (f b) -> p f b", b=NB)
                pt_b = (
                    pt[:]
                    .rearrange("p (f one) -> p f one", one=1)
                    .broadcast_to([P, F, NB])
                )
                thr_b = (
                    thr[:]
                    .rearrange("p (one b) -> p one b", one=1)
                    .broadcast_to([P, F, NB])
                )
                nc.vector.tensor_tensor(ge3, pt_b, thr_b, ALU.is_ge)
                ds_b = (
                    ds[:]
                    .rearrange("p (one b) -> p one b", one=1)
                    .broadcast_to([P, F, NB])
                )
                dm_b = (
                    dm[:]
                    .rearrange("p (one b) -> p one b", one=1)
                    .broadcast_to([P, F, NB])
                )
                prods = tpool.tile([P, F * NB], f32, tag="prods")
                nc.vector.tensor_tensor(
                    prods[:].rearrange("p (f b) -> p f b", b=NB), ge3, ds_b, ALU.mult
                )
                s_acc = tpool.tile([P, F], f32, tag="s_acc")
                nc.vector.tensor_reduce(
                    s_acc[:], prods[:].rearrange("p (f b) -> p f b", b=NB),
                    mybir.AxisListType.X, ALU.add,
                )
                prodm = tpool.tile([P, F * NB], f32, tag="prodm")
                nc.vector.tensor_tensor(
                    prodm[:].rearrange("p (f b) -> p f b", b=NB), ge3, dm_b, ALU.mult
                )
                m_acc = tpool.tile([P, F], f32, tag="m_acc")
                nc.vector.tensor_reduce(
                    m_acc[:], prodm[:].rearrange("p (f b) -> p f b", b=NB),
                    mybir.AxisListType.X, ALU.add,
                )

                # u = 1 + eps - s*pt ;  y = u^m = exp(m * ln(u))
                nspt = tpool.tile([P, F], f32, tag="nspt")
                nc.vector.scalar_tensor_tensor(
                    nspt[:], s_acc[:], -1.0, pt[:], ALU.mult, ALU.mult
                )
                u = tpool.tile([P, F], f32, tag="u")
                nc.vector.tensor_scalar(u[:], nspt[:], 1.0 + EPS, None, ALU.add)
                v = tpool.tile([P, F], f32, tag="v")
                nc.scalar.activation(v[:], u[:], AF.Ln)
                w_ = tpool.tile([P, F], f32, tag="w")
                nc.vector.tensor_mul(w_[:], v[:], m_acc[:])
                y = tpool.tile([P, F], f32, tag="y")
                nc.scalar.activation(y[:], w_[:], AF.Exp)

                # per-partition partial of sum_j y*logpt (negated on host)
                prod = tpool.tile([P, F], f32, tag="prod")
                nc.vector.tensor_mul(prod[:], y[:], logpt[:])
                rs = tpool.tile([P, 1], f32, tag=f"rowsum{h}", name=f"rowsum{h}")
                nc.vector.tensor_reduce(
                    rs[:], prod[:], mybir.AxisListType.X, ALU.add
                )
                rowsums.append(rs)

            def do_slot(slot, xtile, off, is_act, eo=None):
                h, col = slot_part(slot)
                if not is_act:
                    edum = epool.tile([P, C], f16, tag="edum")
                    nc.vector.tensor_scalar(
                        edum[:], eo[:], 1.0, None,
                        ALU.mult, ALU.add,
                        accum_out=sumexp[h][:, col : col + 1],
                    )
                # rows are target-sorted, so this slot's targets all sit
                # inside a static window: the gather scans only it
                lo = slot_lo[slot]
                so = spool.tile([P, w], f32, tag="so")
                nc.vector.scalar_tensor_tensor(
                    so[:],
                    iota_t[:, lo : lo + w],
                    tmap_t[:, slot : slot + 1],
                    xtile[:, off + lo : off + lo + w],
                    ALU.is_equal,
                    ALU.mult,
                    accum_out=xt[h][:, col : col + 1],
                )

            done_halves = set()
            for kind, slot in schedule:
                if kind in ("A", "D"):
                    if slot in early:
                        xtile = early[slot]
                    else:
                        xtile = iopool.tile([P, C], f32, tag="xtile")
                        nc.sync.dma_start(xtile[:], x3[slot, :, :])
                    eo = epool.tile([P, C], f16, tag="eo")
                    if kind == "A":
                        h, col = slot_part(slot)
                        nc.scalar.activation(
                            eo[:], xtile[:], AF.Exp,
                            accum_out=sumexp[h][:, col : col + 1],
                        )
                        do_slot(slot, xtile, 0, True)
                    else:
                        nc.scalar.activation(eo[:], xtile[:], AF.Exp)
                        do_slot(slot, xtile, 0, False, eo=eo)
                    hi = slot
                else:  # D2: slots (slot, slot+1) in one DMA + one ACTIVATE
                    xtile = iopool.tile([P, 2 * C], f32, tag="xtile2")
                    nc.sync.dma_start(
                        xtile[:].rearrange("p (q c) -> p q c", q=2),
                        x4[slot // 2, :, :, :],
                    )
                    eo2 = epool.tile([P, 2 * C], f16, tag="eo2")
                    nc.scalar.activation(eo2[:], xtile[:], AF.Exp)
                    for q in range(2):
                        s2 = slot + q
                        h, col = slot_part(s2)
                        edum = epool.tile([P, C], f16, tag="edum")
                        nc.vector.tensor_scalar(
                            edum[:], eo2[:, q * C : (q + 1) * C], 1.0, None,
                            ALU.mult, ALU.add,
                            accum_out=sumexp[h][:, col : col + 1],
                        )
                        lo = slot_lo[s2]
                        so = spool.tile([P, w], f32, tag="so")
                        nc.vector.scalar_tensor_tensor(
                            so[:],
                            iota_t[:, lo : lo + w],
                            tmap_t[:, s2 : s2 + 1],
                            xtile[:, q * C + lo : q * C + lo + w],
                            ALU.is_equal,
                            ALU.mult,
                            accum_out=xt[h][:, col : col + 1],
                        )
                    hi = slot + 1
                for hh in range(NPART - 1):
                    if hi >= TAIL_BOUNDS[hh + 1] - 1 and hh not in done_halves:
                        done_halves.add(hh)
                        tail_half(hh)  # overlaps the rest of the stream
            tail_half(NPART - 1)

            total = rowsums[0]
            for q in range(1, NPART):
                tq = tpool.tile([P, 1], f32, tag=f"tq{q}", name=f"tq{q}")
                nc.vector.tensor_add(tq[:], total[:], rowsums[q][:])
                total = tq
            ones = tpool.tile([P, 1], f32, tag="ones")
            nc.vector.memset(ones[:], 1.0)
            ps = ppool.tile([1, 1], f32, tag="ps")
            nc.tensor.matmul(ps[:], ones[:], total[:], start=True, stop=True)
            res = tpool.tile([1, 1], f32, tag="res")
            nc.scalar.copy(res[:], ps[:])
            nc.sync.dma_start(out[:], res[:])

    _split_excess_waits(nc, mybir, max_waits=1)
    return nc


_NC_CACHE = {}


def _get_nc(w):
    if w not in _NC_CACHE:
        _NC_CACHE[w] = _build(w)
    return _NC_CACHE[w]


def _make_in_maps(input, target, gammas, w):
    inp = np.ascontiguousarray(np.asarray(input, dtype=np.float32))
    tgt = np.asarray(target).astype(np.int64)
    gam = np.asarray(gammas, dtype=np.float32)
    assert inp.shape == (N, C) and tgt.shape == (N,) and gam.shape == (NUM_BINS,)

    iota_const = np.ascontiguousarray(
        np.broadcast_to(np.arange(C, dtype=np.float32), (P, C))
    )
    gb_const = np.ascontiguousarray(np.broadcast_to(gam, (P, NUM_BINS)))
    slot_lo = np.asarray(_slot_lo(w), dtype=np.int64)

    in_maps = []
    for i in range(NCORES):
        tshard = tgt[NSHARD * i : NSHARD * (i + 1)]
        # sort rows by target; rank r -> slot r//P, partition r%P, so each
        # slot's 128 targets fall inside its static gather window
        order = np.argsort(tshard, kind="stable")
        tsorted = tshard[order]
        by_slot = tsorted.reshape(R, P)  # [slot, partition]
        lo = slot_lo[:, None]
        if not np.all((by_slot >= lo) & (by_slot <= lo + (w - 1))):
            return None  # caller falls back to full-width windows
        shard = np.ascontiguousarray(inp[NSHARD * i : NSHARD * (i + 1)][order])
        tmap = np.ascontiguousarray(by_slot.T).astype(np.float32)  # [P, R]
        in_maps.append(
            {"x": shard, "tmap": tmap, "iota": iota_const, "gb": gb_const}
        )
    return in_maps


def kernel(input, target, gammas, _trace=False, _tmpdir=None):
    from concourse.bass_utils import run_bass_kernel_spmd

    in_maps = _make_in_maps(input, target, gammas, W)
    w = W
    if in_maps is None:
        # pathological target distribution: use full-width gather windows
        w = C
        in_maps = _make_in_maps(input, target, gammas, w)
        assert in_maps is not None  # w == C always satisfies the window check

    res = run_bass_kernel_spmd(
        _get_nc(w),
        in_maps,
        core_ids=list(range(NCORES)),
        trace=_trace,
        tmpdir=_tmpdir,
    )
    partials = [float(res.results[i]["out"][0, 0]) for i in range(NCORES)]
    total = -np.float32(np.sum(np.asarray(partials, dtype=np.float32)))
    if _trace:
        kernel._last_result = res
    return np.array(total, dtype=np.float32)
